# revision 4
# baseline (speedup 1.0000x reference)
"""Multi-head attention (B=4, S=2048, D=1024, H=16, HD=64) on 8 TRN2 cores.

Sharding: batch (4) x head-halves (2) -> 8 cores; core c handles batch c//2
and heads [8*(c%2), 8*(c%2)+8) (512 columns of every projection). No
cross-core communication.

Per-core kernel (Tile framework, bf16 matmuls / fp32 accumulation):
  - X (Q/K/V batch slices) DMA'd in fp32, cast to bf16, transposed to
    d-major layout with the xbar transpose DMA. Ingest is K-first so
    attention (and the exp engines) start as early as possible.
  - Projections on TensorE: qT/kT produced feature-major [head-pair cols,
    toks], V produced token-major with a ones column interleaved per head
    ([v_h | 1] is the PV stationary operand, so softmax denominators fall
    out of the PV matmul for free).
  - scores^T = kT.T @ qT per 128-token k-chunk, two heads packed in the
    128x128 PE array via row tiling (each head contracts over 64 rows).
  - softmax exp split across TWO engines: head a (and optionally some of
    head b) uses a Schraudolph-style exp on VectorE -- one fused
    tensor_scalar op computes i16 = int(s * 128/(8*ln2) + bias) whose bit
    pattern IS bf16 exp(s/8); head b uses ScalarE's exact exp. The split
    roughly halves the softmax critical path; the denominator uses the
    same approximated P so normalization stays consistent.
  - out^T[hd+1, q] accumulates over k-chunks in PSUM; row 64 is the denom.
  - finalize: PE-transpose to token-major, reciprocal of the transposed
    denominators straight from PSUM, per-token scale, DMA out.

Schedule notes: scores for group g+1 are emitted before PV of group g
(software pipeline); within a group all head-a score matmuls precede
head-b's so head-a's exp (VectorE) starts as early as possible -- the
next group's scores WAR-wait on that exp's PSUM read. K is ingested
first (then Q0, V), so attention starts ~20us into the kernel; later Q
blocks are ingested between attention block iterations. ktv/qt pools
are double-buffered so back-to-back reps overlap.
"""

import numpy as np

import concourse.bass as bass
import concourse.tile as tile
from concourse import mybir
from concourse.masks import make_identity

B, S, D_IN, D_MODEL, H = 4, 2048, 1024, 1024, 16
HD = D_MODEL // H  # 64
N_CORES = 8
COLS = 512  # per-core projection columns (8 heads)
NPAIR = 4  # head pairs per core
NKC = S // 128  # 16 k-chunks
NDC = D_IN // 128  # 8 d_in chunks
QB = 512  # q block
NQB = S // QB  # 4

F32 = mybir.dt.float32
BF16 = mybir.dt.bfloat16
I16 = mybir.dt.int16
EXP = mybir.ActivationFunctionType.Exp
MUL = mybir.AluOpType.mult
ADD = mybir.AluOpType.add
COPY = mybir.ActivationFunctionType.Copy

# Schraudolph exp in bf16-bit domain: exp(s/8) ~= bitcast_bf16(int16(
#   s * (2^7/ln2)/8 + (127*2^7 + C) )).  C tuned for softmax output error.
EXP_A8 = 128.0 / np.log(2.0) / 8.0
EXP_B = 16256.0 - 7.5


def _fixup_multi_waits(nc):
    """Split >cap sync waits per instruction into preceding same-engine NoOps.

    This walrus build rejects more than 1 sync wait command per instruction
    (2 for EventSemaphore); Tile's drain/backedge paths can attach one wait
    per live semaphore to a single Drain.
    """
    for fn in nc.m.functions:
        for block in fn.blocks:
            insts = block.instructions
            i = 0
            while i < len(insts):
                inst = insts[i]
                si = inst.sync_info
                cap = 2 if isinstance(inst, mybir.InstEventSemaphore) else 1
                if si is not None and len(si.on_wait) > cap:
                    waits = list(si.on_wait)
                    keep, extra = waits[:cap], waits[cap:]
                    inst.sync_info = mybir.SyncInfo(
                        on_wait=keep, on_update=list(si.on_update)
                    )
                    nops = [
                        mybir.InstNoOp(
                            name=f"{inst.name}_xwait{j}",
                            engine=inst.engine,
                            bass_nofuse=True,
                            sync_info=mybir.SyncInfo(on_wait=[w], on_update=[]),
                        )
                        for j, w in enumerate(extra)
                    ]
                    insts[i:i] = nops
                    i += len(nops)
                i += 1


class _TC(tile.TileContext):
    def __exit__(self, *args):
        ret = super().__exit__(*args)
        _fixup_multi_waits(self.nc)
        return ret


def build_core_program(
    fixup_waits: bool = True,
    time_reps: int = 1,
    prologue_only: bool = False,
    ablate: str = "",
    n_dve_a: int = 8,  # head-a groups (of 8) whose exp runs on DVE
    n_dve_b: int = 0,  # head-b groups (of 8) whose exp runs on DVE
    fin_v3: bool = False,
    exp_split: bool = False,
    glen_cfg: int = 1,
    sc_tags: bool = False,
    fin_act_copy: bool = True,
    sc_bufs: int = 4,
    depth: int = 2,  # score-group prefetch depth (software pipeline)
) -> bass.Bass:
    tc_cls = _TC if fixup_waits else tile.TileContext
    nc = bass.Bass()
    xq = nc.dram_tensor("xq", [S, D_IN], F32, kind="ExternalInput")
    xk = nc.dram_tensor("xk", [S, D_IN], F32, kind="ExternalInput")
    xv = nc.dram_tensor("xv", [S, D_IN], F32, kind="ExternalInput")
    wq = nc.dram_tensor("wq", [D_IN, COLS], F32, kind="ExternalInput")
    wk = nc.dram_tensor("wk", [D_IN, COLS], F32, kind="ExternalInput")
    wv = nc.dram_tensor("wv", [D_IN, COLS], F32, kind="ExternalInput")
    bqp = nc.dram_tensor("bqp", [128, NPAIR], F32, kind="ExternalInput")
    bkp = nc.dram_tensor("bkp", [128, NPAIR], F32, kind="ExternalInput")
    bvb = nc.dram_tensor("bvb", [128, COLS], F32, kind="ExternalInput")
    out = nc.dram_tensor("out", [S, COLS], F32, kind="ExternalOutput")

    from contextlib import ExitStack

    with tc_cls(nc) as tc:
        with ExitStack() as ctx:
            ec = ctx.enter_context
            cpool = ec(tc.tile_pool(name="const", bufs=1))
            wpool = ec(tc.tile_pool(name="wsb", bufs=1))
            xstage_pool = ec(tc.tile_pool(name="xstage", bufs=4))
            xbf_pool = ec(tc.tile_pool(name="xbf", bufs=4))
            xtq_pool = ec(tc.tile_pool(name="xtq", bufs=2))
            xtkv_pool = ec(tc.tile_pool(name="xtkv", bufs=2))
            ktv_pool = ec(tc.tile_pool(name="ktv", bufs=2))
            qt_pool = ec(tc.tile_pool(name="qt", bufs=2))
            pt_pool = ec(tc.tile_pool(name="pt", bufs=6))
            outt_pool = ec(tc.tile_pool(name="outt", bufs=2))
            small_pool = ec(tc.tile_pool(name="small", bufs=2))
            ob_pool = ec(tc.tile_pool(name="ob", bufs=2))
            sc_ps = ec(tc.tile_pool(name="psc", bufs=sc_bufs, space="PSUM"))
            proj_ps = ec(tc.tile_pool(name="pproj", bufs=2, space="PSUM"))
            pv_ps = ec(tc.tile_pool(name="ppv", bufs=2, space="PSUM"))
            # --- constants ---
            idn = cpool.tile([128, 128], F32)
            make_identity(nc, idn[:])
            bqp_sb = cpool.tile([128, NPAIR], F32, tag="bqp")
            bkp_sb = cpool.tile([128, NPAIR], F32, tag="bkp")
            bvb_sb = cpool.tile([128, COLS], F32, tag="bvb")
            nc.sync.dma_start(bqp_sb[:], bqp[:])
            nc.sync.dma_start(bkp_sb[:], bkp[:])
            nc.sync.dma_start(bvb_sb[:], bvb[:])

            for _rep in range(time_reps):
                # --- weights: SWDGE cast-DMA fp32->bf16 (wk first: K-block
                # projections are the first PE work of a rep) ---
                w_sb = {}
                for name, wd in (("k", wk), ("q", wq), ("v", wv)):
                    wsb = wpool.tile([128, NDC, COLS], BF16, tag=f"w{name}")
                    nc.gpsimd.dma_start(
                        wsb[:], wd.rearrange("(c p) n -> p c n", p=128)
                    )
                    w_sb[name] = wsb

                def load_xt_block(xdram, dest, tc0, ntc, alt=False):
                    # dest[:, dc, (tc-tc0)*128 + t] = x[tc*128 + t, dc*128 + dp]
                    # fp32 loads on the SP HWDGE ring, casts on GpSimd (DVE
                    # helps for the first blocks, before attention needs it),
                    # then grouped xbar transposes (also SP ring).
                    xbs = []
                    for tci in range(tc0, tc0 + ntc):
                        st = xstage_pool.tile([128, D_IN], F32, tag="xst")
                        nc.sync.dma_start(st[:], xdram[tci * 128 : (tci + 1) * 128, :])
                        xb = xbf_pool.tile([128, D_IN], BF16, tag="xbf")
                        eng = nc.vector if alt else nc.gpsimd
                        eng.tensor_copy(xb[:], st[:])
                        xbs.append(xb)
                    for i, xb in enumerate(xbs):
                        o = i * 128
                        nc.sync.dma_start(
                            dest[:, :, o : o + 128], xb[:], transpose=True
                        )

                kT_blk, v_blk = [None] * 4, [None] * 4
                qT_all = [None] * 4

                def ingest_q(blk):
                    xtqb = xtq_pool.tile([128, NDC, QB], BF16, tag="xtq")
                    load_xt_block(xq, xtqb, blk * 4, 4)
                    qT_blk = qt_pool.tile([128, NPAIR, QB], BF16, tag="qt")
                    qT_all[blk] = qT_blk
                    for p in range(NPAIR):
                        ps = proj_ps.tile([128, 512], F32, tag="proj")
                        for dc in range(NDC):
                            nc.tensor.matmul(
                                ps[:],
                                w_sb["q"][:, dc, p * 128 : (p + 1) * 128],
                                xtqb[:, dc, :],
                                start=(dc == 0),
                                stop=(dc == NDC - 1),
                            )
                        nc.vector.tensor_scalar_add(
                            qT_blk[:, p, :], ps[:], bqp_sb[:, p : p + 1]
                        )

                def ingest_k(blk):
                    xtk = xtkv_pool.tile([128, NDC, 512], BF16, tag="xtkv")
                    load_xt_block(xk, xtk, blk * 4, 4, alt=(blk == 0))
                    kT = ktv_pool.tile([128, NPAIR, 512], BF16, tag=f"kT{blk}")
                    kT_blk[blk] = kT
                    for p in range(NPAIR):
                        ps = proj_ps.tile([128, 512], F32, tag="proj")
                        for dc in range(NDC):
                            nc.tensor.matmul(
                                ps[:],
                                w_sb["k"][:, dc, p * 128 : (p + 1) * 128],
                                xtk[:, dc, :],
                                start=(dc == 0),
                                stop=(dc == NDC - 1),
                            )
                        nc.vector.tensor_scalar_add(
                            kT[:, p, :], ps[:], bkp_sb[:, p : p + 1]
                        )

                def ingest_v(blk):
                    xtv = xtkv_pool.tile([128, NDC, 512], BF16, tag="xtkv")
                    load_xt_block(xv, xtv, blk * 4, 4)
                    vb = ktv_pool.tile([128, 4, 8, HD + 1], BF16, tag=f"v{blk}")
                    v_blk[blk] = vb
                    nc.gpsimd.memset(vb[:, :, :, HD : HD + 1], 1.0)
                    for tci in range(4):
                        ps = proj_ps.tile([128, 512], F32, tag="proj")
                        for dc in range(NDC):
                            nc.tensor.matmul(
                                ps[:],
                                xtv[:, dc, tci * 128 : (tci + 1) * 128],
                                w_sb["v"][:, dc, :],
                                start=(dc == 0),
                                stop=(dc == NDC - 1),
                            )
                        nc.vector.tensor_add(
                            vb[:, tci, :, 0:HD],
                            ps[:].rearrange("p (h d) -> p h d", h=8),
                            bvb_sb[:].rearrange("p (h d) -> p h d", h=8),
                        )

                GROUPS = [
                    (glen_cfg * g, glen_cfg) for g in range(NKC // glen_cfg)
                ]
                # which (group, head) exps run on DVE vs ScalarE
                ngrp = NKC // glen_cfg
                dve_a_groups = set(range(n_dve_a * ngrp // 8))
                dve_b_groups = set(range(ngrp - n_dve_b * ngrp // 8, ngrp))

                def exp_dve(dst_bf16, src_psum, ew):
                    # split per 512-chunk: halves the exp->PV latency
                    nchunk = max(1, ew // 512) if exp_split else 1
                    cw = ew // nchunk
                    for ci in range(nchunk):
                        nc.vector.tensor_scalar(
                            dst_bf16[:, ci * cw : (ci + 1) * cw].bitcast(I16),
                            src_psum[:, ci * cw : (ci + 1) * cw],
                            float(EXP_A8),
                            float(EXP_B),
                            op0=MUL,
                            op1=ADD,
                        )

                def scores_grp(j, p, g, qT_blk):
                    kc0, glen = GROUPS[g]
                    scA = sc_ps.tile([128, glen * 512], F32, tag="sca" if sc_tags else "sc")
                    scB = sc_ps.tile([128, glen * 512], F32, tag="scb" if sc_tags else "sc")
                    for u in range(glen):
                        kc = kc0 + u
                        nc.tensor.matmul(
                            scA[:, u * 512 : (u + 1) * 512],
                            kT_blk[kc // 4][0:64, p, (kc % 4) * 128 : (kc % 4 + 1) * 128],
                            qT_blk[0:64, p, :],
                            start=True,
                            stop=True,
                        )
                    for u in range(glen):
                        kc = kc0 + u
                        nc.tensor.matmul(
                            scB[:, u * 512 : (u + 1) * 512],
                            kT_blk[kc // 4][64:128, p, (kc % 4) * 128 : (kc % 4 + 1) * 128],
                            qT_blk[64:128, p, :],
                            tile_position=(64, 0),
                            start=True,
                            stop=True,
                        )
                    ew = glen * 512 // 8 if ablate == "exp_narrow" else glen * 512
                    pta = pt_pool.tile([128, glen * 512], BF16, tag="pt")
                    ptb = pt_pool.tile([128, glen * 512], BF16, tag="pt")
                    if g in dve_a_groups:
                        exp_dve(pta[:, 0:ew], scA, ew)
                    else:
                        nc.scalar.activation(
                            pta[:, 0:ew], scA[:, 0:ew], EXP, scale=0.125
                        )
                    if g in dve_b_groups:
                        exp_dve(ptb[:, 0:ew], scB, ew)
                    else:
                        nc.scalar.activation(
                            ptb[:, 0:ew], scB[:, 0:ew], EXP, scale=0.125
                        )
                    return pta, ptb

                def pv_grp(p, g, pta, ptb, psO_a, psO_b):
                    kc0, glen = GROUPS[g]
                    for pt_h, psO, hoff in ((pta, psO_a, 0), (ptb, psO_b, 1)):
                        for u in range(glen):
                            kc = kc0 + u
                            if ablate == "pv_lite" and kc not in (0, NKC - 1):
                                continue
                            nc.tensor.matmul(
                                psO[0:65, :],
                                v_blk[kc // 4][:, kc % 4, 2 * p + hoff, :],
                                pt_h[:, u * 512 : (u + 1) * 512],
                                start=(kc == 0) or ablate == "pv_lite",
                                stop=(kc == NKC - 1) or ablate == "pv_lite",
                            )

                def finalize_pair(j, p, psO_a, psO_b):
                    if not fin_v3:
                        return finalize_pair_v2(j, p, psO_a, psO_b)
                    # normalize in out^T space: recip rows -> gpsimd broadcast
                    # -> gpsimd mul; then 4 PE transposes into ONE psum bank
                    # and a single DVE evacuation copy.
                    outT = outt_pool.tile([128, QB], F32, tag="outT")
                    nc.vector.tensor_copy(outT[0:64, :], psO_a[0:64, :])
                    nc.vector.tensor_copy(outT[64:128, :], psO_b[0:64, :])
                    ra = small_pool.tile([1, QB], F32, tag="dena")
                    rb = small_pool.tile([1, QB], F32, tag="denb")
                    nc.vector.reciprocal(ra[:], psO_a[64:65, :])
                    nc.vector.reciprocal(rb[:], psO_b[64:65, :])
                    bc = outt_pool.tile([128, QB], F32, tag="bc")
                    nc.gpsimd.partition_broadcast(bc[0:64, :], ra[:], channels=64)
                    nc.gpsimd.partition_broadcast(bc[64:128, :], rb[:], channels=64)
                    outN = outt_pool.tile([128, QB], F32, tag="outN")
                    nc.gpsimd.tensor_mul(outN[:], outT[:], bc[:])
                    ob = ob_pool.tile([128, 4, 128], F32, tag="ob")
                    tp = proj_ps.tile([128, 512], F32, tag="proj")
                    for tci in range(4):
                        nc.tensor.transpose(
                            tp[:, tci * 128 : (tci + 1) * 128],
                            outN[:, tci * 128 : (tci + 1) * 128],
                            idn[:],
                        )
                    nc.vector.tensor_copy(
                        ob[:].rearrange("p t c -> p (t c)"), tp[:]
                    )
                    nc.gpsimd.dma_start(
                        out[j * QB : (j + 1) * QB, p * 128 : (p + 1) * 128]
                        .rearrange("(tb t) c -> t tb c", t=128),
                        ob[:],
                    )

                def finalize_pair_v2(j, p, psO_a, psO_b):
                    # PSUM evacuation copies run on ScalarE (Copy is in every
                    # act table-set): keeps the DVE FIFO shallow so the next
                    # pair's Schraudolph exps aren't queued behind finalize.
                    cpeng = nc.scalar if fin_act_copy else nc.vector
                    outT = outt_pool.tile([128, QB], F32, tag="outT")
                    if fin_act_copy:
                        cpeng.activation(outT[0:64, :], psO_a[0:64, :], COPY)
                        cpeng.activation(outT[64:128, :], psO_b[0:64, :], COPY)
                    else:
                        cpeng.tensor_copy(outT[0:64, :], psO_a[0:64, :])
                        cpeng.tensor_copy(outT[64:128, :], psO_b[0:64, :])
                    dena = small_pool.tile([1, QB], F32, tag="dena")
                    denb = small_pool.tile([1, QB], F32, tag="denb")
                    nc.vector.tensor_copy(dena[:], psO_a[64:65, :])
                    nc.vector.tensor_copy(denb[:], psO_b[64:65, :])
                    ob = ob_pool.tile([128, 4, 128], F32, tag="ob")
                    for tci in range(4):
                        tp = proj_ps.tile([128, 512], F32, tag="proj")
                        nc.tensor.transpose(
                            tp[:, 0:128], outT[:, tci * 128 : (tci + 1) * 128], idn[:]
                        )
                        nc.tensor.transpose(
                            tp[:, 128:129],
                            dena[:, tci * 128 : (tci + 1) * 128],
                            idn[0:1, 0:1],
                        )
                        nc.tensor.transpose(
                            tp[:, 129:130],
                            denb[:, tci * 128 : (tci + 1) * 128],
                            idn[0:1, 0:1],
                        )
                        rT = small_pool.tile([128, 2], F32, tag="rT")
                        nc.vector.reciprocal(rT[:], tp[:, 128:130])
                        nc.vector.tensor_scalar_mul(
                            ob[:, tci, 0:64], tp[:, 0:64], rT[:, 0:1]
                        )
                        nc.vector.tensor_scalar_mul(
                            ob[:, tci, 64:128], tp[:, 64:128], rT[:, 1:2]
                        )
                    nc.gpsimd.dma_start(
                        out[j * QB : (j + 1) * QB, p * 128 : (p + 1) * 128]
                        .rearrange("(tb t) c -> t tb c", t=128),
                        ob[:],
                    )

                def attention_pair(j, p, qT_blk):
                    # software-pipelined: emit scores one group ahead of PV so
                    # the in-order PE stream never blocks on the current
                    # group's exp.
                    psO_a = pv_ps.tile([128, 512], F32, tag="pv", name=f"psOa_{j}_{p}")
                    psO_b = pv_ps.tile([128, 512], F32, tag="pv", name=f"psOb_{j}_{p}")
                    ng = len(GROUPS)
                    pts = {}
                    for gp in range(min(depth, ng)):
                        pts[gp] = scores_grp(j, p, gp, qT_blk)
                    for g in range(ng):
                        if g + depth < ng:
                            pts[g + depth] = scores_grp(j, p, g + depth, qT_blk)
                        pta, ptb = pts.pop(g)
                        pv_grp(p, g, pta, ptb, psO_a, psO_b)
                    finalize_pair(j, p, psO_a, psO_b)

                # --- K-first ingest; attention interleaved with Q ingest ---
                ingest_k(0)
                ingest_q(0)
                ingest_v(0)
                ingest_k(1)
                ingest_v(1)
                ingest_k(2)
                ingest_v(2)
                ingest_k(3)
                ingest_v(3)

                if prologue_only:
                    dummy = ob_pool.tile([128, 128], F32, tag="ob")
                    nc.vector.tensor_copy(dummy[:], kT_blk[3][:, 3, 0:128])
                    nc.gpsimd.dma_start(out[0:128, 0:128], dummy[:])
                else:
                    for j in range(NQB):
                        if j + 1 < NQB:
                            ingest_q(j + 1)
                        for p in range(NPAIR):
                            attention_pair(j, p, qT_all[j])

    return nc


def _shard_inputs(Q, V, K, wq, bq, wk, bk, wv, bv):
    in_maps = []
    for c in range(N_CORES):
        b, half = c // 2, c % 2
        lo = half * COLS
        bq_s, bk_s, bv_s = bq[lo : lo + COLS], bk[lo : lo + COLS], bv[lo : lo + COLS]
        in_maps.append(
            {
                "xq": np.ascontiguousarray(Q[b]),
                "xk": np.ascontiguousarray(K[b]),
                "xv": np.ascontiguousarray(V[b]),
                "wq": np.ascontiguousarray(wq[:, lo : lo + COLS]),
                "wk": np.ascontiguousarray(wk[:, lo : lo + COLS]),
                "wv": np.ascontiguousarray(wv[:, lo : lo + COLS]),
                "bqp": np.ascontiguousarray(bq_s.reshape(NPAIR, 128).T),
                "bkp": np.ascontiguousarray(bk_s.reshape(NPAIR, 128).T),
                "bvb": np.ascontiguousarray(
                    np.broadcast_to(bv_s, (128, COLS))
                ),
            }
        )
    return in_maps


class SpmdRunner:
    """Compile a Bass program once; run it on 8 cores via PJRT with timing.

    Mirrors bass2jax.run_bass_via_pjrt's multi-core path but keeps the jitted
    executable so repeat executions don't re-trace/re-compile.
    """

    def __init__(self, nc: bass.Bass, n_cores: int = 8):
        import jax
        from jax.sharding import Mesh, PartitionSpec
        from jax.experimental.shard_map import shard_map
        from concourse import bass2jax
        from concourse.bass2jax import _bass_exec_p, install_neuronx_cc_hook

        install_neuronx_cc_hook()
        self.nc = nc
        self.n_cores = n_cores
        self._jax = jax
        self._PartitionSpec = PartitionSpec

        in_names, out_names, out_avals, zero_outs = [], [], [], []
        partition_name = (
            nc.partition_id_tensor.name if nc.partition_id_tensor else None
        )
        for alloc in nc.m.functions[0].allocations:
            if not isinstance(alloc, mybir.MemoryLocationSet):
                continue
            name = alloc.memorylocations[0].name
            if alloc.kind == "ExternalInput":
                if name != partition_name:
                    in_names.append(name)
            elif alloc.kind == "ExternalOutput":
                out_names.append(name)
                shape = tuple(alloc.tensor_shape)
                dtype = mybir.dt.np(alloc.dtype)
                out_avals.append(jax.core.ShapedArray(shape, dtype))
                zero_outs.append(np.zeros(shape, dtype))

        self.in_names = in_names
        self.out_names = out_names
        self.out_avals = out_avals
        self.zero_outs = zero_outs
        n_params = len(in_names)
        n_outs = len(out_avals)
        all_in_names = list(in_names) + list(out_names)
        if partition_name is not None:
            all_in_names.append(partition_name)

        donate = tuple(range(n_params, n_params + n_outs))

        def _body(*args):
            operands = list(args)
            if partition_name is not None:
                operands.append(bass2jax.partition_id_tensor())
            outs = _bass_exec_p.bind(
                *operands,
                out_avals=tuple(out_avals),
                in_names=tuple(all_in_names),
                out_names=tuple(out_names),
                lowering_input_output_aliases=(),
                sim_require_finite=True,
                sim_require_nnan=True,
                nc=nc,
            )
            return tuple(outs)

        devices = jax.devices()[:n_cores]
        self.mesh = Mesh(np.asarray(devices), ("core",))
        in_specs = (PartitionSpec("core"),) * (n_params + n_outs)
        out_specs = (PartitionSpec("core"),) * len(out_names)
        self.sharded = jax.jit(
            shard_map(
                _body,
                mesh=self.mesh,
                in_specs=in_specs,
                out_specs=out_specs,
                check_rep=False,
            ),
            donate_argnums=donate,
            keep_unused=True,
        )

    def run(self, in_maps, iters: int = 1):
        """Returns (results_per_core, best_iter_seconds)."""
        import time as _time

        jax = self._jax
        from jax.sharding import NamedSharding

        sh = NamedSharding(self.mesh, self._PartitionSpec("core"))
        per_core = [
            [np.asarray(m[name]) for name in self.in_names] for m in in_maps
        ]
        concat_in = [
            np.concatenate([per_core[c][i] for c in range(self.n_cores)], axis=0)
            for i in range(len(self.in_names))
        ]
        concat_in = [jax.device_put(a, sh) for a in concat_in]
        for a in concat_in:
            a.block_until_ready()
        times = []
        out_arrs = None
        for _ in range(iters):
            concat_zeros = [
                jax.device_put(
                    np.zeros((self.n_cores * z.shape[0], *z.shape[1:]), z.dtype),
                    sh,
                )
                for z in self.zero_outs
            ]
            for z in concat_zeros:
                z.block_until_ready()
            t0 = _time.perf_counter()
            out_arrs = self.sharded(*concat_in, *concat_zeros)
            for o in out_arrs:
                o.block_until_ready()
            t1 = _time.perf_counter()
            times.append(t1 - t0)
        results = [
            {
                name: np.asarray(out_arrs[i]).reshape(
                    self.n_cores, *self.out_avals[i].shape
                )[c]
                for i, name in enumerate(self.out_names)
            }
            for c in range(self.n_cores)
        ]
        return results, min(times)


_RUNNER = None


def _get_runner():
    global _RUNNER
    if _RUNNER is None:
        _RUNNER = SpmdRunner(build_core_program(), n_cores=N_CORES)
    return _RUNNER


def kernel(**inputs) -> np.ndarray:
    inputs = {k: np.asarray(v) for k, v in inputs.items()}
    in_maps = _shard_inputs(**inputs)
    runner = _get_runner()
    results, _ = runner.run(in_maps, iters=1)
    out = np.empty((B, S, D_MODEL), np.float32)
    for c in range(N_CORES):
        b, half = c // 2, c % 2
        out[b, :, half * COLS : (half + 1) * COLS] = results[c]["out"]
    return out



# revision 24
# speedup vs baseline: 1.0725x; 1.0725x over previous
"""Multi-head attention (B=4, S=2048, D=1024, H=16, HD=64) on 8 TRN2 cores.

Sharding: batch (4) x head-halves (2) -> 8 cores; core c handles batch c//2
and heads [8*(c%2), 8*(c%2)+8) (512 columns of every projection). No
cross-core communication.

Per-core kernel (Tile framework, bf16 matmuls / fp32 accumulation):
  - X (Q/K/V batch slices) DMA'd in fp32, cast to bf16, transposed to
    d-major layout with the xbar transpose DMA. Ingest is K-first so
    attention (and the exp engines) start as early as possible.
  - Projections on TensorE: qT/kT produced feature-major [head-pair cols,
    toks], V produced token-major with a ones column interleaved per head
    ([v_h | 1] is the PV stationary operand, so softmax denominators fall
    out of the PV matmul for free).
  - scores^T = kT.T @ qT per 128-token k-chunk, two heads packed in the
    128x128 PE array via row tiling (each head contracts over 64 rows).
  - softmax exp split across TWO engines: head a (and optionally some of
    head b) uses a Schraudolph-style exp on VectorE -- one fused
    tensor_scalar op computes i16 = int(s * 128/(8*ln2) + bias) whose bit
    pattern IS bf16 exp(s/8); head b uses ScalarE's exact exp. The split
    roughly halves the softmax critical path; the denominator uses the
    same approximated P so normalization stays consistent.
  - out^T[hd+1, q] accumulates over k-chunks in PSUM; row 64 is the denom.
  - finalize: PE-transpose to token-major, reciprocal of the transposed
    denominators straight from PSUM, per-token scale, DMA out.

Schedule notes: scores for group g+1 are emitted before PV of group g
(software pipeline); within a group all head-a score matmuls precede
head-b's so head-a's exp (VectorE) starts as early as possible -- the
next group's scores WAR-wait on that exp's PSUM read. K is ingested
first (then Q0, V), so attention starts ~20us into the kernel; later Q
blocks are ingested between attention block iterations. ktv/qt pools
are double-buffered so back-to-back reps overlap.
"""

import numpy as np

import concourse.bass as bass
import concourse.tile as tile
from concourse import mybir
from concourse.masks import make_identity

B, S, D_IN, D_MODEL, H = 4, 2048, 1024, 1024, 16
HD = D_MODEL // H  # 64
N_CORES = 8
COLS = 512  # per-core projection columns (8 heads)
NPAIR = 4  # head pairs per core
NKC = S // 128  # 16 k-chunks
NDC = D_IN // 128  # 8 d_in chunks
QB = 512  # q block
NQB = S // QB  # 4

F32 = mybir.dt.float32
BF16 = mybir.dt.bfloat16
I16 = mybir.dt.int16
EXP = mybir.ActivationFunctionType.Exp
MUL = mybir.AluOpType.mult
ADD = mybir.AluOpType.add
COPY = mybir.ActivationFunctionType.Copy

# Schraudolph exp in bf16-bit domain: exp(s/8) ~= bitcast_bf16(int16(
#   s * (2^7/ln2)/8 + (127*2^7 + C) )).  C tuned for softmax output error.
EXP_A8 = 128.0 / np.log(2.0) / 8.0
EXP_B = 16256.0 - 7.5


def _fixup_multi_waits(nc):
    """Split >cap sync waits per instruction into preceding same-engine NoOps.

    This walrus build rejects more than 1 sync wait command per instruction
    (2 for EventSemaphore); Tile's drain/backedge paths can attach one wait
    per live semaphore to a single Drain.
    """
    for fn in nc.m.functions:
        for block in fn.blocks:
            insts = block.instructions
            i = 0
            while i < len(insts):
                inst = insts[i]
                si = inst.sync_info
                cap = 2 if isinstance(inst, mybir.InstEventSemaphore) else 1
                if si is not None and len(si.on_wait) > cap:
                    waits = list(si.on_wait)
                    keep, extra = waits[:cap], waits[cap:]
                    inst.sync_info = mybir.SyncInfo(
                        on_wait=keep, on_update=list(si.on_update)
                    )
                    nops = [
                        mybir.InstNoOp(
                            name=f"{inst.name}_xwait{j}",
                            engine=inst.engine,
                            bass_nofuse=True,
                            sync_info=mybir.SyncInfo(on_wait=[w], on_update=[]),
                        )
                        for j, w in enumerate(extra)
                    ]
                    insts[i:i] = nops
                    i += len(nops)
                i += 1


class _TC(tile.TileContext):
    def __exit__(self, *args):
        ret = super().__exit__(*args)
        _fixup_multi_waits(self.nc)
        return ret


def build_core_program(
    fixup_waits: bool = True,
    time_reps: int = 1,
    prologue_only: bool = False,
    ablate: str = "",
    n_dve_a: int = 8,  # head-a groups (of 8) whose exp runs on DVE
    n_dve_b: int = 0,  # head-b groups (of 8) whose exp runs on DVE
    fin_v3: bool = False,
    exp_split: bool = False,
    glen_cfg: int = 1,
    sc_tags: bool = False,
    fin_act_copy: bool = True,
    sc_bufs: int = 4,
    depth: int = 2,  # score-group prefetch depth (software pipeline)
    pv_half: bool = False,  # PV as 64-row half-chunks — RUNTIME-INVALID on HW
    proj_half: bool = False,  # (accum group across tile_position switch
    #                            wedges the device; keep False)
    bias_act: bool = True,  # K-proj bias evac on Act (DVE relief)
    ring_split: bool = True,  # X stage loads on Act HWDGE ring
    fin_v5: bool = True,  # finalize: den-in-copy, sc_ps transposes, big DMA
    n_a_act: int = 0,  # groups per pair whose head-a exp moves DVE->Act
) -> bass.Bass:
    tc_cls = _TC if fixup_waits else tile.TileContext
    nc = bass.Bass()
    xq = nc.dram_tensor("xq", [S, D_IN], F32, kind="ExternalInput")
    xk = nc.dram_tensor("xk", [S, D_IN], F32, kind="ExternalInput")
    xv = nc.dram_tensor("xv", [S, D_IN], F32, kind="ExternalInput")
    wq = nc.dram_tensor("wq", [D_IN, COLS], F32, kind="ExternalInput")
    wk = nc.dram_tensor("wk", [D_IN, COLS], F32, kind="ExternalInput")
    wv = nc.dram_tensor("wv", [D_IN, COLS], F32, kind="ExternalInput")
    bqp = nc.dram_tensor("bqp", [128, NPAIR], F32, kind="ExternalInput")
    bkp = nc.dram_tensor("bkp", [128, NPAIR], F32, kind="ExternalInput")
    bvb = nc.dram_tensor("bvb", [128, COLS], F32, kind="ExternalInput")
    out = nc.dram_tensor("out", [S, COLS], F32, kind="ExternalOutput")

    from contextlib import ExitStack

    with tc_cls(nc) as tc:
        with ExitStack() as ctx:
            ec = ctx.enter_context
            cpool = ec(tc.tile_pool(name="const", bufs=1))
            wpool = ec(tc.tile_pool(name="wsb", bufs=1))
            xstage_pool = ec(tc.tile_pool(name="xstage", bufs=4))
            xbf_pool = ec(tc.tile_pool(name="xbf", bufs=4))
            xtq_pool = ec(tc.tile_pool(name="xtq", bufs=2))
            xtkv_pool = ec(tc.tile_pool(name="xtkv", bufs=2))
            ktv_pool = ec(tc.tile_pool(name="ktv", bufs=2))
            qt_pool = ec(tc.tile_pool(name="qt", bufs=2))
            pt_pool = ec(tc.tile_pool(name="pt", bufs=6))
            outt_pool = ec(tc.tile_pool(name="outt", bufs=2))
            small_pool = ec(tc.tile_pool(name="small", bufs=2))
            ob_pool = ec(tc.tile_pool(name="ob", bufs=2))
            sc_ps = ec(tc.tile_pool(name="psc", bufs=sc_bufs, space="PSUM"))
            proj_ps = ec(tc.tile_pool(name="pproj", bufs=2, space="PSUM"))
            pv_ps = ec(tc.tile_pool(name="ppv", bufs=2, space="PSUM"))
            # --- constants ---
            idn = cpool.tile([128, 128], F32)
            make_identity(nc, idn[:])
            bqp_sb = cpool.tile([128, NPAIR], F32, tag="bqp")
            bkp_sb = cpool.tile([128, NPAIR], F32, tag="bkp")
            bvb_sb = cpool.tile([128, COLS], F32, tag="bvb")
            nc.sync.dma_start(bqp_sb[:], bqp[:])
            nc.sync.dma_start(bkp_sb[:], bkp[:])
            nc.sync.dma_start(bvb_sb[:], bvb[:])

            for _rep in range(time_reps):
                # --- weights: SWDGE cast-DMA fp32->bf16 (wk first: K-block
                # projections are the first PE work of a rep) ---
                w_sb = {}
                for name, wd in (("k", wk), ("q", wq), ("v", wv)):
                    wsb = wpool.tile([128, NDC, COLS], BF16, tag=f"w{name}")
                    nc.gpsimd.dma_start(
                        wsb[:], wd.rearrange("(c p) n -> p c n", p=128)
                    )
                    w_sb[name] = wsb

                def load_xt_block(xdram, dest, tc0, ntc, alt=False):
                    # dest[:, dc, (tc-tc0)*128 + t] = x[tc*128 + t, dc*128 + dp]
                    # fp32 loads on the Act HWDGE ring (ring_split) so they
                    # don't serialize behind the xbar transposes on SP; casts
                    # on GpSimd (DVE helps for the first blocks, before
                    # attention needs it).
                    ld_eng = nc.scalar if ring_split else nc.sync
                    xbs = []
                    for tci in range(tc0, tc0 + ntc):
                        st = xstage_pool.tile([128, D_IN], F32, tag="xst")
                        ld_eng.dma_start(st[:], xdram[tci * 128 : (tci + 1) * 128, :])
                        xb = xbf_pool.tile([128, D_IN], BF16, tag="xbf")
                        eng = nc.vector if alt else nc.gpsimd
                        eng.tensor_copy(xb[:], st[:])
                        xbs.append(xb)
                    for i, xb in enumerate(xbs):
                        o = i * 128
                        nc.sync.dma_start(
                            dest[:, :, o : o + 128], xb[:], transpose=True
                        )

                def proj_matmuls(ps, lhs_full, rhs_full, nchunk):
                    # accumulate nchunk 128-deep chunks into ps; 64-row halves
                    # at alternating tile positions stream ~2.3 cols/cycle
                    if proj_half:
                        for i in range(2 * nchunk):
                            dc, half = i // 2, i % 2
                            r0 = 64 * half
                            nc.tensor.matmul(
                                ps,
                                lhs_full(dc)[r0 : r0 + 64],
                                rhs_full(dc)[r0 : r0 + 64],
                                tile_position=(r0, 0),
                                start=(i == 0),
                                stop=(i == 2 * nchunk - 1),
                            )
                    else:
                        for dc in range(nchunk):
                            nc.tensor.matmul(
                                ps,
                                lhs_full(dc),
                                rhs_full(dc),
                                start=(dc == 0),
                                stop=(dc == nchunk - 1),
                            )

                kT_blk, v_blk = [None] * 4, [None] * 4
                qT_all = [None] * 4

                def ingest_q(blk):
                    xtqb = xtq_pool.tile([128, NDC, QB], BF16, tag="xtq")
                    load_xt_block(xq, xtqb, blk * 4, 4)
                    qT_blk = qt_pool.tile([128, NPAIR, QB], BF16, tag="qt")
                    qT_all[blk] = qT_blk
                    for p in range(NPAIR):
                        ps = proj_ps.tile([128, 512], F32, tag="proj")
                        proj_matmuls(
                            ps[:],
                            lambda dc, p=p: w_sb["q"][:, dc, p * 128 : (p + 1) * 128],
                            lambda dc: xtqb[:, dc, :],
                            NDC,
                        )
                        nc.vector.tensor_scalar_add(
                            qT_blk[:, p, :], ps[:], bqp_sb[:, p : p + 1]
                        )

                def ingest_k(blk):
                    xtk = xtkv_pool.tile([128, NDC, 512], BF16, tag="xtkv")
                    load_xt_block(xk, xtk, blk * 4, 4, alt=(blk == 0))
                    kT = ktv_pool.tile([128, NPAIR, 512], BF16, tag=f"kT{blk}")
                    kT_blk[blk] = kT
                    for p in range(NPAIR):
                        ps = proj_ps.tile([128, 512], F32, tag="proj")
                        proj_matmuls(
                            ps[:],
                            lambda dc, p=p: w_sb["k"][:, dc, p * 128 : (p + 1) * 128],
                            lambda dc: xtk[:, dc, :],
                            NDC,
                        )
                        if bias_act:
                            # K bias-evac on Act: out = Identity(in + bias_row)
                            # (Identity lives in the exp_and_friends act table,
                            # so no table switch vs the exp instructions)
                            nc.scalar.activation(
                                kT[:, p, :], ps[:],
                                mybir.ActivationFunctionType.Identity,
                                bias=bkp_sb[:, p : p + 1],
                            )
                        else:
                            nc.vector.tensor_scalar_add(
                                kT[:, p, :], ps[:], bkp_sb[:, p : p + 1]
                            )

                def ingest_v(blk):
                    xtv = xtkv_pool.tile([128, NDC, 512], BF16, tag="xtkv")
                    load_xt_block(xv, xtv, blk * 4, 4)
                    vb = ktv_pool.tile([128, 4, 8, HD + 1], BF16, tag=f"v{blk}")
                    v_blk[blk] = vb
                    nc.gpsimd.memset(vb[:, :, :, HD : HD + 1], 1.0)
                    for tci in range(4):
                        ps = proj_ps.tile([128, 512], F32, tag="proj")
                        proj_matmuls(
                            ps[:],
                            lambda dc, tci=tci: xtv[:, dc, tci * 128 : (tci + 1) * 128],
                            lambda dc: w_sb["v"][:, dc, :],
                            NDC,
                        )
                        nc.vector.tensor_add(
                            vb[:, tci, :, 0:HD],
                            ps[:].rearrange("p (h d) -> p h d", h=8),
                            bvb_sb[:].rearrange("p (h d) -> p h d", h=8),
                        )

                GROUPS = [
                    (glen_cfg * g, glen_cfg) for g in range(NKC // glen_cfg)
                ]
                # which (group, head) exps run on DVE vs ScalarE
                ngrp = NKC // glen_cfg
                dve_a_groups = set(range(n_dve_a * ngrp // 8))
                dve_b_groups = set(range(ngrp - n_dve_b * ngrp // 8, ngrp))
                # head-a exps moved to Act for load balance (exp is the
                # per-engine bottleneck once PE runs half-chunk matmuls)
                a_act_groups = (
                    set(range(ngrp // n_a_act - 1, ngrp, ngrp // n_a_act))
                    if n_a_act > 0
                    else set()
                )

                def exp_dve(dst_bf16, src_psum, ew):
                    # split per 512-chunk: halves the exp->PV latency
                    nchunk = max(1, ew // 512) if exp_split else 1
                    cw = ew // nchunk
                    for ci in range(nchunk):
                        nc.vector.tensor_scalar(
                            dst_bf16[:, ci * cw : (ci + 1) * cw].bitcast(I16),
                            src_psum[:, ci * cw : (ci + 1) * cw],
                            float(EXP_A8),
                            float(EXP_B),
                            op0=MUL,
                            op1=ADD,
                        )

                def scores_grp(j, p, g, qT_blk):
                    kc0, glen = GROUPS[g]
                    scA = sc_ps.tile([128, glen * 512], F32, tag="sca" if sc_tags else "sc")
                    scB = sc_ps.tile([128, glen * 512], F32, tag="scb" if sc_tags else "sc")
                    for u in range(glen):
                        kc = kc0 + u
                        nc.tensor.matmul(
                            scA[:, u * 512 : (u + 1) * 512],
                            kT_blk[kc // 4][0:64, p, (kc % 4) * 128 : (kc % 4 + 1) * 128],
                            qT_blk[0:64, p, :],
                            start=True,
                            stop=True,
                        )
                    for u in range(glen):
                        kc = kc0 + u
                        nc.tensor.matmul(
                            scB[:, u * 512 : (u + 1) * 512],
                            kT_blk[kc // 4][64:128, p, (kc % 4) * 128 : (kc % 4 + 1) * 128],
                            qT_blk[64:128, p, :],
                            tile_position=(64, 0),
                            start=True,
                            stop=True,
                        )
                    ew = glen * 512 // 8 if ablate == "exp_narrow" else glen * 512
                    pta = pt_pool.tile([128, glen * 512], BF16, tag="pt")
                    ptb = pt_pool.tile([128, glen * 512], BF16, tag="pt")
                    ngrp_pair = NKC // glen_cfg
                    a_act = (g % ngrp_pair) in a_act_groups
                    if g in dve_a_groups and not a_act:
                        exp_dve(pta[:, 0:ew], scA, ew)
                    else:
                        nc.scalar.activation(
                            pta[:, 0:ew], scA[:, 0:ew], EXP, scale=0.125
                        )
                    if g in dve_b_groups:
                        exp_dve(ptb[:, 0:ew], scB, ew)
                    else:
                        nc.scalar.activation(
                            ptb[:, 0:ew], scB[:, 0:ew], EXP, scale=0.125
                        )
                    return pta, ptb

                def pv_grp(p, g, pta, ptb, psO_a, psO_b):
                    kc0, glen = GROUPS[g]
                    for pt_h, psO, hoff in ((pta, psO_a, 0), (ptb, psO_b, 1)):
                        for u in range(glen):
                            kc = kc0 + u
                            if ablate == "pv_lite" and kc not in (0, NKC - 1):
                                continue
                            vop = v_blk[kc // 4][:, kc % 4, 2 * p + hoff, :]
                            pop = pt_h[:, u * 512 : (u + 1) * 512]
                            if pv_half:
                                for half in range(2):
                                    r0 = 64 * half
                                    nc.tensor.matmul(
                                        psO[0:65, :],
                                        vop[r0 : r0 + 64],
                                        pop[r0 : r0 + 64],
                                        tile_position=(r0, 0),
                                        start=(kc == 0 and half == 0)
                                        or ablate == "pv_lite",
                                        stop=(kc == NKC - 1 and half == 1)
                                        or ablate == "pv_lite",
                                    )
                            else:
                                nc.tensor.matmul(
                                    psO[0:65, :],
                                    vop,
                                    pop,
                                    start=(kc == 0) or ablate == "pv_lite",
                                    stop=(kc == NKC - 1) or ablate == "pv_lite",
                                )

                def finalize_pair(j, p, psO_a, psO_b):
                    if not fin_v3:
                        return finalize_pair_v2(j, p, psO_a, psO_b)
                    # normalize in out^T space: recip rows -> gpsimd broadcast
                    # -> gpsimd mul; then 4 PE transposes into ONE psum bank
                    # and a single DVE evacuation copy.
                    outT = outt_pool.tile([128, QB], F32, tag="outT")
                    nc.vector.tensor_copy(outT[0:64, :], psO_a[0:64, :])
                    nc.vector.tensor_copy(outT[64:128, :], psO_b[0:64, :])
                    ra = small_pool.tile([1, QB], F32, tag="dena")
                    rb = small_pool.tile([1, QB], F32, tag="denb")
                    nc.vector.reciprocal(ra[:], psO_a[64:65, :])
                    nc.vector.reciprocal(rb[:], psO_b[64:65, :])
                    bc = outt_pool.tile([128, QB], F32, tag="bc")
                    nc.gpsimd.partition_broadcast(bc[0:64, :], ra[:], channels=64)
                    nc.gpsimd.partition_broadcast(bc[64:128, :], rb[:], channels=64)
                    outN = outt_pool.tile([128, QB], F32, tag="outN")
                    nc.gpsimd.tensor_mul(outN[:], outT[:], bc[:])
                    ob = ob_pool.tile([128, 4, 128], F32, tag="ob")
                    tp = proj_ps.tile([128, 512], F32, tag="proj")
                    for tci in range(4):
                        nc.tensor.transpose(
                            tp[:, tci * 128 : (tci + 1) * 128],
                            outN[:, tci * 128 : (tci + 1) * 128],
                            idn[:],
                        )
                    nc.vector.tensor_copy(
                        ob[:].rearrange("p t c -> p (t c)"), tp[:]
                    )
                    nc.gpsimd.dma_start(
                        out[j * QB : (j + 1) * QB, p * 128 : (p + 1) * 128]
                        .rearrange("(tb t) c -> t tb c", t=128),
                        ob[:],
                    )

                def finalize_pair_v2(j, p, psO_a, psO_b):
                    # PSUM evacuation copies run on ScalarE (Copy is in every
                    # act table-set): keeps the DVE FIFO shallow so the next
                    # pair's Schraudolph exps aren't queued behind finalize.
                    cpeng = nc.scalar if fin_act_copy else nc.vector
                    outT = outt_pool.tile([128, QB], F32, tag="outT")
                    if fin_act_copy:
                        cpeng.activation(outT[0:64, :], psO_a[0:64, :], COPY)
                        cpeng.activation(outT[64:128, :], psO_b[0:64, :], COPY)
                    else:
                        cpeng.tensor_copy(outT[0:64, :], psO_a[0:64, :])
                        cpeng.tensor_copy(outT[64:128, :], psO_b[0:64, :])
                    dena = small_pool.tile([1, QB], F32, tag="dena")
                    denb = small_pool.tile([1, QB], F32, tag="denb")
                    nc.vector.tensor_copy(dena[:], psO_a[64:65, :])
                    nc.vector.tensor_copy(denb[:], psO_b[64:65, :])
                    ob = ob_pool.tile([128, 4, 128], F32, tag="ob")
                    for tci in range(4):
                        tp = proj_ps.tile([128, 512], F32, tag="proj")
                        nc.tensor.transpose(
                            tp[:, 0:128], outT[:, tci * 128 : (tci + 1) * 128], idn[:]
                        )
                        nc.tensor.transpose(
                            tp[:, 128:129],
                            dena[:, tci * 128 : (tci + 1) * 128],
                            idn[0:1, 0:1],
                        )
                        nc.tensor.transpose(
                            tp[:, 129:130],
                            denb[:, tci * 128 : (tci + 1) * 128],
                            idn[0:1, 0:1],
                        )
                        rT = small_pool.tile([128, 2], F32, tag="rT")
                        nc.vector.reciprocal(rT[:], tp[:, 128:130])
                        nc.vector.tensor_scalar_mul(
                            ob[:, tci, 0:64], tp[:, 0:64], rT[:, 0:1]
                        )
                        nc.vector.tensor_scalar_mul(
                            ob[:, tci, 64:128], tp[:, 64:128], rT[:, 1:2]
                        )
                    if ablate != "no_outdma":
                        nc.gpsimd.dma_start(
                            out[j * QB : (j + 1) * QB, p * 128 : (p + 1) * 128]
                            .rearrange("(tb t) c -> t tb c", t=128),
                            ob[:],
                        )

                def finalize_pair_v5(j, p, psO_a, psO_b, obf):
                    # Evacuate psO including the denominator row (65-row Act
                    # copies), PE-transpose 65-col chunks (denominators ride
                    # along as column 64/129), DVE reciprocal + per-token
                    # scale into the j-block output tile obf. tp tiles come
                    # from sc_ps so finalize never contends with the
                    # interleaved Q-projection's proj_ps.
                    outTa = outt_pool.tile([128, QB], F32, tag="outT")
                    outTb = outt_pool.tile([128, QB], F32, tag="outT")
                    nc.scalar.activation(outTa[0:65, :], psO_a[0:65, :], COPY)
                    nc.scalar.activation(outTb[0:65, :], psO_b[0:65, :], COPY)
                    for tci in range(4):
                        tp = sc_ps.tile([128, 512], F32, tag="sc")
                        nc.tensor.transpose(
                            tp[:, 0:65],
                            outTa[0:65, tci * 128 : (tci + 1) * 128],
                            idn[0:65, 0:65],
                        )
                        nc.tensor.transpose(
                            tp[:, 65:130],
                            outTb[0:65, tci * 128 : (tci + 1) * 128],
                            idn[0:65, 0:65],
                        )
                        rT = small_pool.tile([128, 2], F32, tag="rT")
                        nc.vector.reciprocal(rT[:], tp[:, 64:130:65])
                        nc.vector.tensor_scalar_mul(
                            obf[:, tci, p, 0:64], tp[:, 0:64], rT[:, 0:1]
                        )
                        nc.vector.tensor_scalar_mul(
                            obf[:, tci, p, 64:128], tp[:, 65:129], rT[:, 1:2]
                        )

                def finalize_lite(j, p, psO_a, psO_b):
                    # ablation: minimal evacuation, no transposes/recip/outdma
                    acc = ob_pool.tile([128, 512], F32, tag="ob")
                    acc2 = ob_pool.tile([128, 512], F32, tag="ob")
                    nc.vector.tensor_copy(acc[0:65, :], psO_a[0:65, :])
                    nc.scalar.activation(acc2[0:65, :], psO_b[0:65, :], COPY)

                def attention_pair(j, p, qT_blk, obf=None):
                    # software-pipelined: emit scores one group ahead of PV so
                    # the in-order PE stream never blocks on the current
                    # group's exp.
                    psO_a = pv_ps.tile([128, 512], F32, tag="pv", name=f"psOa_{j}_{p}")
                    psO_b = pv_ps.tile([128, 512], F32, tag="pv", name=f"psOb_{j}_{p}")
                    ng = len(GROUPS)
                    pts = {}
                    for gp in range(min(depth, ng)):
                        pts[gp] = scores_grp(j, p, gp, qT_blk)
                    for g in range(ng):
                        if g + depth < ng:
                            pts[g + depth] = scores_grp(j, p, g + depth, qT_blk)
                        pta, ptb = pts.pop(g)
                        pv_grp(p, g, pta, ptb, psO_a, psO_b)
                    if ablate == "no_fin":
                        finalize_lite(j, p, psO_a, psO_b)
                    elif fin_v5:
                        finalize_pair_v5(j, p, psO_a, psO_b, obf)
                    else:
                        finalize_pair(j, p, psO_a, psO_b)

                # --- K-first ingest; attention interleaved with Q ingest ---
                ingest_k(0)
                ingest_q(0)
                ingest_v(0)
                ingest_k(1)
                ingest_v(1)
                ingest_k(2)
                ingest_v(2)
                ingest_k(3)
                ingest_v(3)

                if prologue_only:
                    dummy = ob_pool.tile([128, 128], F32, tag="ob")
                    nc.vector.tensor_copy(dummy[:], kT_blk[3][:, 3, 0:128])
                    nc.gpsimd.dma_start(out[0:128, 0:128], dummy[:])
                else:
                    for j in range(NQB):
                        if j + 1 < NQB:
                            ingest_q(j + 1)
                        obf = None
                        if fin_v5 and ablate != "no_fin":
                            obf = ob_pool.tile([128, 4, 4, 128], F32, tag="obf")
                        for p in range(NPAIR):
                            attention_pair(j, p, qT_all[j], obf)
                        if obf is not None and ablate != "no_outdma":
                            # one contiguous 1MB DMA per j-block: DRAM runs
                            # are full 2KB rows
                            nc.gpsimd.dma_start(
                                out[j * QB : (j + 1) * QB, :]
                                .rearrange("(tb t) c -> t tb c", t=128),
                                obf[:].rearrange("t tb p c -> t tb (p c)"),
                            )

    return nc


def _shard_inputs(Q, V, K, wq, bq, wk, bk, wv, bv):
    in_maps = []
    for c in range(N_CORES):
        b, half = c // 2, c % 2
        lo = half * COLS
        bq_s, bk_s, bv_s = bq[lo : lo + COLS], bk[lo : lo + COLS], bv[lo : lo + COLS]
        in_maps.append(
            {
                "xq": np.ascontiguousarray(Q[b]),
                "xk": np.ascontiguousarray(K[b]),
                "xv": np.ascontiguousarray(V[b]),
                "wq": np.ascontiguousarray(wq[:, lo : lo + COLS]),
                "wk": np.ascontiguousarray(wk[:, lo : lo + COLS]),
                "wv": np.ascontiguousarray(wv[:, lo : lo + COLS]),
                "bqp": np.ascontiguousarray(bq_s.reshape(NPAIR, 128).T),
                "bkp": np.ascontiguousarray(bk_s.reshape(NPAIR, 128).T),
                "bvb": np.ascontiguousarray(
                    np.broadcast_to(bv_s, (128, COLS))
                ),
            }
        )
    return in_maps


class SpmdRunner:
    """Compile a Bass program once; run it on 8 cores via PJRT with timing.

    Mirrors bass2jax.run_bass_via_pjrt's multi-core path but keeps the jitted
    executable so repeat executions don't re-trace/re-compile.
    """

    def __init__(self, nc: bass.Bass, n_cores: int = 8):
        import jax
        from jax.sharding import Mesh, PartitionSpec
        from jax.experimental.shard_map import shard_map
        from concourse import bass2jax
        from concourse.bass2jax import _bass_exec_p, install_neuronx_cc_hook

        install_neuronx_cc_hook()
        self.nc = nc
        self.n_cores = n_cores
        self._jax = jax
        self._PartitionSpec = PartitionSpec

        in_names, out_names, out_avals, zero_outs = [], [], [], []
        partition_name = (
            nc.partition_id_tensor.name if nc.partition_id_tensor else None
        )
        for alloc in nc.m.functions[0].allocations:
            if not isinstance(alloc, mybir.MemoryLocationSet):
                continue
            name = alloc.memorylocations[0].name
            if alloc.kind == "ExternalInput":
                if name != partition_name:
                    in_names.append(name)
            elif alloc.kind == "ExternalOutput":
                out_names.append(name)
                shape = tuple(alloc.tensor_shape)
                dtype = mybir.dt.np(alloc.dtype)
                out_avals.append(jax.core.ShapedArray(shape, dtype))
                zero_outs.append(np.zeros(shape, dtype))

        self.in_names = in_names
        self.out_names = out_names
        self.out_avals = out_avals
        self.zero_outs = zero_outs
        n_params = len(in_names)
        n_outs = len(out_avals)
        all_in_names = list(in_names) + list(out_names)
        if partition_name is not None:
            all_in_names.append(partition_name)

        donate = tuple(range(n_params, n_params + n_outs))

        def _body(*args):
            operands = list(args)
            if partition_name is not None:
                operands.append(bass2jax.partition_id_tensor())
            outs = _bass_exec_p.bind(
                *operands,
                out_avals=tuple(out_avals),
                in_names=tuple(all_in_names),
                out_names=tuple(out_names),
                lowering_input_output_aliases=(),
                sim_require_finite=True,
                sim_require_nnan=True,
                nc=nc,
            )
            return tuple(outs)

        devices = jax.devices()[:n_cores]
        self.mesh = Mesh(np.asarray(devices), ("core",))
        in_specs = (PartitionSpec("core"),) * (n_params + n_outs)
        out_specs = (PartitionSpec("core"),) * len(out_names)
        self.sharded = jax.jit(
            shard_map(
                _body,
                mesh=self.mesh,
                in_specs=in_specs,
                out_specs=out_specs,
                check_rep=False,
            ),
            donate_argnums=donate,
            keep_unused=True,
        )

    def run(self, in_maps, iters: int = 1):
        """Returns (results_per_core, best_iter_seconds)."""
        import time as _time

        jax = self._jax
        from jax.sharding import NamedSharding

        sh = NamedSharding(self.mesh, self._PartitionSpec("core"))
        per_core = [
            [np.asarray(m[name]) for name in self.in_names] for m in in_maps
        ]
        concat_in = [
            np.concatenate([per_core[c][i] for c in range(self.n_cores)], axis=0)
            for i in range(len(self.in_names))
        ]
        concat_in = [jax.device_put(a, sh) for a in concat_in]
        for a in concat_in:
            a.block_until_ready()
        times = []
        out_arrs = None
        for _ in range(iters):
            concat_zeros = [
                jax.device_put(
                    np.zeros((self.n_cores * z.shape[0], *z.shape[1:]), z.dtype),
                    sh,
                )
                for z in self.zero_outs
            ]
            for z in concat_zeros:
                z.block_until_ready()
            t0 = _time.perf_counter()
            out_arrs = self.sharded(*concat_in, *concat_zeros)
            for o in out_arrs:
                o.block_until_ready()
            t1 = _time.perf_counter()
            times.append(t1 - t0)
        results = [
            {
                name: np.asarray(out_arrs[i]).reshape(
                    self.n_cores, *self.out_avals[i].shape
                )[c]
                for i, name in enumerate(self.out_names)
            }
            for c in range(self.n_cores)
        ]
        return results, min(times)


_RUNNER = None


def _get_runner():
    global _RUNNER
    if _RUNNER is None:
        _RUNNER = SpmdRunner(build_core_program(), n_cores=N_CORES)
    return _RUNNER


def kernel(**inputs) -> np.ndarray:
    inputs = {k: np.asarray(v) for k, v in inputs.items()}
    in_maps = _shard_inputs(**inputs)
    runner = _get_runner()
    results, _ = runner.run(in_maps, iters=1)
    out = np.empty((B, S, D_MODEL), np.float32)
    for c in range(N_CORES):
        b, half = c // 2, c % 2
        out[b, :, half * COLS : (half + 1) * COLS] = results[c]["out"]
    return out

